# revision 6
# baseline (speedup 1.0000x reference)
"""DynamicSparseAttention Trainium2 kernel (8-core SPMD).

Math (exactly equivalent to the dense reference):
  The top-K mask multiplies scores by 0/1 (not -inf), so a masked key
  contributes exp(0)=1 to the softmax sum and v_j to the numerator.
  With p_j = exp(s_j/8) over the K selected keys only:
    denom = sum_top (p_j - 1) + S
    numer = sum_top (p_j - 1) * v_j + sum_all v_j
  so only the K selected K/V rows plus one "sum over all V rows" vector
  are needed -- the S x S dense attention never materializes.

Sharding: the B*S=4096 query rows are split 8 ways (cores 0-3 batch 0,
cores 4-7 batch 1); no cross-core communication. Top-K selection (a
[B,S] GEMV + argpartition) and layout prep run on host; all GEMMs on
device.

Precision: large GEMMs use a bf16 hi/lo split (3 matmul terms, error
~2^-18 -- fp32-grade) at 1 cycle/row instead of native fp32's 4 cycles;
the small attention matmuls stay native fp32; the rank-1 "rest of V"
term (~2% of output magnitude) uses single bf16.

Attention layout: head h lives at (tile i=h//2, partition 32*(h%2)).
Scores for a head pair are ONE matmul: lhsT is a [128, 64]
block-diagonal K^T tile (two 64x32 blocks), rhs is Q^T's d-block
[128, 512], out [64, 512] -- keeps every AP base partition in {0,32,64}
(hardware restriction) and uses the full 128-lane contraction.
"""

import os
import sys
import types

import numpy as np

B, S, D, H, HD, K = 2, 2048, 1024, 16, 64, 32
NC = 8
SB = B * S // NC          # 512 query rows per core
CPB = NC // B             # cores per batch
KB = D // 128             # 128-row contraction blocks
INV_SQRT_HD = 1.0 / 8.0

_cache = {}
last_results = None


def _install_ntff_hook():
    """trace=True under axon needs antenv.axon_hooks, absent on this image.
    Synthesize it from trn_boot's ctypes implementation; degrade silently."""
    if "antenv.axon_hooks" in sys.modules:
        return
    try:
        from trn_agent_boot.trn_boot import _ntff_profile_via_ctypes

        hook = _ntff_profile_via_ctypes("/opt/axon/libaxon_pjrt.so")
        m = types.ModuleType("antenv.axon_hooks")
        m.get_axon_ntff_profile_hook = lambda: hook
        sys.modules["antenv.axon_hooks"] = m
    except Exception:
        pass


def _build():
    import concourse.bacc as bacc
    import concourse.mybir as mybir
    import concourse.tile as tile

    F32 = mybir.dt.float32
    BF16 = mybir.dt.bfloat16
    EXP = mybir.ActivationFunctionType.Exp
    CPY = mybir.ActivationFunctionType.Copy

    nc = bacc.Bacc("TRN2", target_bir_lowering=False, debug=False, num_devices=NC)

    def inp(name, shape, dt=F32):
        return nc.dram_tensor(name, shape, dt, kind="ExternalInput").ap()

    # bf16 hi/lo pairs are stacked as [2, rows, cols] (index 0=hi, 1=lo)
    xT2 = inp("xT2", [2, D, SB], BF16)      # this core's x rows, transposed, split
    wq2 = inp("wq2", [2, D, D], BF16)       # wq.T split; lhsT tile [k, m] = W[m, k]
    wv2 = inp("wv2", [2, D, D], BF16)
    wo2 = inp("wo2", [2, D, D], BF16)
    wkT = inp("wkT", [D, D])                # K path stays native fp32
    xsT32 = inp("xsT32", [D, K])            # selected x rows, transposed (fp32)
    xsr2 = inp("xsr2", [2, D, 64], BF16)    # selected rows 2x replicated, split
    xsum2 = inp("xsum2", [2, D, 1], BF16)   # sum over all x rows, split
    bq8 = inp("bq8", [128, KB])             # bq[d] as [128,8]: col c = bq[128c:128c+128]
    bk8 = inp("bk8", [128, KB])
    bo8 = inp("bo8", [128, KB])
    bvr = inp("bvr", [64, D])               # bv broadcast to 64 rows
    bva = inp("bva", [1, D])                # S * bv
    BOc = inp("BOc", [64, 64])              # 2x block-diag 32x32 ones, fp32
    outT = nc.dram_tensor("outT", [D, SB], F32, kind="ExternalOutput").ap()

    with tile.TileContext(nc) as tc:
        with (
            tc.tile_pool(name="wqo", bufs=16) as wqo_pool,   # wq pair then wo pair
            tc.tile_pool(name="wkp", bufs=8) as wk_pool,     # wk fp32
            tc.tile_pool(name="wvp", bufs=16) as wv_pool,    # wv pair
            tc.tile_pool(name="xa", bufs=16) as xa_pool,     # xT pair, then att pair
            tc.tile_pool(name="xsr", bufs=16) as xsr_pool,
            tc.tile_pool(name="xs32", bufs=8) as xs32_pool,
            tc.tile_pool(name="xsum", bufs=16) as xsum_pool,
            tc.tile_pool(name="qt", bufs=8) as qtp,          # Q^T resident
            tc.tile_pool(name="kbd", bufs=8) as kbdp,        # block-diag K^T
            tc.tile_pool(name="cst", bufs=1) as cst,         # consts + V tiles
            tc.tile_pool(name="sma", bufs=2) as sma,         # softmax intermediates
            tc.tile_pool(name="pq", bufs=2, space="PSUM") as pq,
            tc.tile_pool(name="pk", bufs=2, space="PSUM") as pk,
            tc.tile_pool(name="ps", bufs=2, space="PSUM") as psc,
            tc.tile_pool(name="pa", bufs=2, space="PSUM") as pa,
        ):
            # ---- resident loads ----
            def load_pair(pool, src, cols, tag):
                tiles = []
                for half in range(2):
                    for kb in range(KB):
                        t = pool.tile([128, cols], BF16, tag=tag)
                        nc.sync.dma_start(
                            t[:], src[half, 128 * kb:128 * kb + 128, :])
                        tiles.append(t)
                return tiles[:KB], tiles[KB:]

            xh, xl = load_pair(xa_pool, xT2, SB, "xa")
            wqh, wql = load_pair(wqo_pool, wq2, D, "w")
            wk = []
            for kb in range(KB):
                t = wk_pool.tile([128, D], F32, tag="wk")
                nc.sync.dma_start(t[:], wkT[128 * kb:128 * kb + 128, :])
                wk.append(t)
            xs32 = []
            for kb in range(KB):
                t = xs32_pool.tile([128, K], F32, tag="xs32")
                nc.sync.dma_start(t[:], xsT32[128 * kb:128 * kb + 128, :])
                xs32.append(t)
            xsrh, xsrl = load_pair(xsr_pool, xsr2, 64, "xsr")
            xsumh, xsuml = load_pair(xsum_pool, xsum2, 1, "xsum")

            def const(name, src, shape, dt=F32):
                t = cst.tile(shape, dt, tag=name)
                nc.sync.dma_start(t[:], src)
                return t

            bq_sb = const("bq", bq8, [128, KB])
            bk_sb = const("bk", bk8, [128, KB])
            bo_sb = const("bo", bo8, [128, KB])
            bvr_sb = const("bvr", bvr, [64, D])
            bva_sb = const("bva", bva, [1, D])
            bo_c = const("BOc", BOc, [64, 64])

            def mm3(out, lh, ll, rh, rl, first, last):
                """out += lh.T@rh + lh.T@rl + ll.T@rh  (bf16 hi/lo 3-term)"""
                nc.tensor.matmul(out, lh, rh, start=first, stop=False)
                nc.tensor.matmul(out, lh, rl, start=False, stop=False)
                nc.tensor.matmul(out, ll, rh, start=False, stop=last)

            # ---- Q^T = wq @ x^T (+bq), bf16x3 ----
            QT = []
            for db in range(KB):
                ps = pq.tile([128, SB], F32, tag="pp")
                cs = slice(128 * db, 128 * db + 128)
                for kb in range(KB):
                    mm3(ps[:], wqh[kb][:, cs], wql[kb][:, cs], xh[kb][:],
                        xl[kb][:], kb == 0, kb == KB - 1)
                t = qtp.tile([128, SB], F32, tag="qt")
                nc.vector.tensor_scalar_add(t[:], ps[:], bq_sb[:, db:db + 1])
                QT.append(t)

            # ---- block-diagonal K^T tiles (native fp32 GEMM).
            # KBD[db] is [128, 64]: [0:64, 0:32] = K^T of head 2db (+bk),
            # [64:128, 32:64] = K^T of head 2db+1 (+bk), zeros elsewhere.
            KBD = []
            for db in range(KB):
                ps = pk.tile([128, K], F32, tag="pk")
                for kb in range(KB):
                    nc.tensor.matmul(ps[:], wk[kb][:, 128 * db:128 * db + 128],
                                     xs32[kb][:], start=kb == 0, stop=kb == KB - 1)
                t = kbdp.tile([128, 2 * K], F32, tag="kbd")
                nc.gpsimd.memset(t[:], 0.0)
                nc.vector.tensor_scalar_add(t[0:64, 0:K], ps[0:64, :],
                                            bk_sb[0:64, db:db + 1])
                nc.vector.tensor_scalar_add(t[64:128, K:2 * K], ps[64:128, :],
                                            bk_sb[64:128, db:db + 1])
                KBD.append(t)

            # ---- V for selected keys, 2x partition-replicated, bf16x3:
            # VR[32r+j, d] = v_sel[j, d] (+bv), r in {0,1}
            wvh, wvl = load_pair(wv_pool, wv2, D, "wv")
            VR = cst.tile([64, D], F32, tag="VR")
            for dh in range(2):
                cs = slice(512 * dh, 512 * dh + 512)
                ps = pk.tile([64, 512], F32, tag="pk")
                for kb in range(KB):
                    mm3(ps[:], xsrh[kb][:], xsrl[kb][:], wvh[kb][:, cs],
                        wvl[kb][:, cs], kb == 0, kb == KB - 1)
                nc.vector.tensor_add(VR[:, cs], ps[:], bvr_sb[:, cs])

            # ---- V_all = (sum_s x_s) @ wv^T + S*bv (bf16x3: with only K=32
            # keys selected most softmax weight mass sits on the "rest" term,
            # so it needs full precision), then replicate to partitions 0/32
            VAr = cst.tile([1, D], F32, tag="VAr")
            for dh in range(2):
                cs = slice(512 * dh, 512 * dh + 512)
                ps = pk.tile([1, 512], F32, tag="pk")
                for kb in range(KB):
                    mm3(ps[:], xsumh[kb][:], xsuml[kb][:], wvh[kb][:, cs],
                        wvl[kb][:, cs], kb == 0, kb == KB - 1)
                nc.vector.tensor_add(VAr[:, cs], ps[:], bva_sb[:, cs])
            VA = cst.tile([64, D], F32, tag="VA")
            for r in range(2):
                nc.sync.dma_start(VA[32 * r:32 * r + 1, :], VAr[:])

            # ---- scores -> p-1 -> 1/denom -> weights (native fp32).
            # Tile i holds heads (2i, 2i+1): head h at partitions 32*(h%2).
            PN = []   # (p-1)/denom, fp32 [64, SB]
            IV = []   # 1/denom, fp32 (rows within a 32-group all equal)
            for i in range(KB):
                ps = psc.tile([64, SB], F32, tag="ps")
                nc.tensor.matmul(ps[:], KBD[i][:], QT[i][:], start=True, stop=True)
                p = sma.tile([64, SB], F32, tag="p")
                nc.scalar.activation(p[:], ps[:], EXP, scale=INV_SQRT_HD)
                pm1 = sma.tile([64, SB], F32, tag="pm1")
                nc.vector.tensor_scalar_add(pm1[:], p[:], -1.0)
                dn = psc.tile([64, SB], F32, tag="ps")
                nc.tensor.matmul(dn[:], bo_c[:], pm1[:], start=True, stop=True)
                dns = sma.tile([64, SB], F32, tag="dns")
                nc.vector.tensor_scalar_add(dns[:], dn[:], float(S))
                iv = wv_pool.tile([64, SB], F32, tag="wv")
                nc.vector.reciprocal(iv[:], dns[:])
                pn = wv_pool.tile([64, SB], F32, tag="wv")
                nc.vector.tensor_mul(pn[:], pm1[:], iv[:])
                PN.append(pn)
                IV.append(iv)

            # ---- att^T per head pair: att^T[hd, s] = V^T pn + VA/denom ----
            ATT = []  # bf16 hi/lo pairs
            for i in range(KB):  # heads (2i, 2i+1)
                ps = pa.tile([128, SB], F32, tag="pa")
                for h in (2 * i, 2 * i + 1):
                    r = 32 * (h % 2)
                    o = ps[64 * (h % 2):64 * (h % 2) + 64, :]
                    nc.tensor.matmul(
                        o, VR[r:r + 32, 64 * h:64 * h + 64],
                        PN[i][r:r + 32, :], start=True, stop=False)
                    nc.tensor.matmul(
                        o, VA[r:r + 1, 64 * h:64 * h + 64],
                        IV[i][r:r + 1, :], start=False, stop=True)
                th = xa_pool.tile([128, SB], BF16, tag="xa")
                nc.scalar.activation(th[:], ps[:], CPY)
                tl = xa_pool.tile([128, SB], BF16, tag="xa")
                nc.vector.tensor_sub(tl[:], ps[:], th[:])
                ATT.append((th, tl))

            # ---- out^T = wo @ att^T (+bo), bf16x3 ----
            woh, wol = load_pair(wqo_pool, wo2, D, "w")
            for db in range(KB):
                ps = pq.tile([128, SB], F32, tag="pp")
                cs = slice(128 * db, 128 * db + 128)
                for kb in range(KB):
                    mm3(ps[:], woh[kb][:, cs], wol[kb][:, cs], ATT[kb][0][:],
                        ATT[kb][1][:], kb == 0, kb == KB - 1)
                t = qtp.tile([128, SB], F32, tag="qt")
                nc.vector.tensor_scalar_add(t[:], ps[:], bo_sb[:, db:db + 1])
                nc.sync.dma_start(outT[128 * db:128 * db + 128, :], t[:])

    nc.compile()
    return nc


def _split(a):
    import ml_dtypes

    hi = a.astype(ml_dtypes.bfloat16)
    lo = (a - hi.astype(np.float32)).astype(ml_dtypes.bfloat16)
    return np.ascontiguousarray(np.stack([hi, lo]))


def kernel(x, wq, bq, wk, bk, wv, bv, wo, bo, ws, bs):
    global last_results
    import ml_dtypes
    from concourse.bass_utils import run_bass_kernel_spmd

    if os.environ.get("BASS_TRACE"):
        _install_ntff_hook()

    x = np.asarray(x, dtype=np.float32)
    wqT, wkT, wvT, woT = (np.ascontiguousarray(np.asarray(w, np.float32).T)
                          for w in (wq, wk, wv, wo))
    bq, bk, bv, bo = (np.asarray(b, np.float32) for b in (bq, bk, bv, bo))
    ws = np.asarray(ws, np.float32)
    bs = np.float32(bs)

    # host prep: importance -> top-K selection -> gather (GEMV + argpartition)
    imp = x.astype(np.float64) @ ws.astype(np.float64) + float(bs)   # [B,S]
    top = np.sort(np.argpartition(imp, S - K, axis=1)[:, -K:], axis=1)
    xsel = np.take_along_axis(x, top[:, :, None], axis=1)            # [B,K,D]
    xsum = x.astype(np.float64).sum(axis=1).astype(np.float32)       # [B,D]

    if "nc" not in _cache:
        _cache["nc"] = _build()
    nc = _cache["nc"]

    blockones = np.zeros((64, 64), np.float32)
    for g in range(2):
        blockones[32 * g:32 * g + 32, 32 * g:32 * g + 32] = 1.0

    def btile(b_):
        return np.ascontiguousarray(b_.reshape(KB, 128).T)

    wq2, wv2, wo2 = _split(wqT), _split(wvT), _split(woT)
    in_maps = []
    for c in range(NC):
        b = c // CPB
        r0 = (c % CPB) * SB
        xsT = np.ascontiguousarray(xsel[b].T)                        # [D, K]
        in_maps.append({
            "xT2": _split(np.ascontiguousarray(x[b, r0:r0 + SB, :].T)),
            "wq2": wq2, "wv2": wv2, "wo2": wo2, "wkT": wkT,
            "xsT32": xsT,
            "xsr2": _split(np.tile(xsT, (1, 2))),
            "xsum2": _split(np.ascontiguousarray(xsum[b][:, None])),
            "bq8": btile(bq), "bk8": btile(bk), "bo8": btile(bo),
            "bvr": np.ascontiguousarray(np.tile(bv[None, :], (64, 1))),
            "bva": np.ascontiguousarray((float(S) * bv)[None, :]),
            "BOc": blockones,
        })

    res = run_bass_kernel_spmd(nc, in_maps, list(range(NC)))
    last_results = res

    out = np.empty((B, S, D), np.float32)
    for c in range(NC):
        b = c // CPB
        r0 = (c % CPB) * SB
        out[b, r0:r0 + SB, :] = res.results[c]["outT"].T
    return out


# revision 7
# speedup vs baseline: 1.1460x; 1.1460x over previous
"""DynamicSparseAttention Trainium2 kernel (8-core SPMD).

Math (exactly equivalent to the dense reference):
  The top-K mask multiplies scores by 0/1 (not -inf), so a masked key
  contributes exp(0)=1 to the softmax sum and v_j to the numerator.
  With p_j = exp(s_j/8) over the K selected keys only:
    denom = sum_top (p_j - 1) + S
    numer = sum_top (p_j - 1) * v_j + sum_all v_j
  so only the K selected K/V rows plus one "sum over all V rows" vector
  are needed -- the S x S dense attention never materializes.

Sharding: the B*S=4096 query rows are split 8 ways (cores 0-3 batch 0,
cores 4-7 batch 1); no cross-core communication. Top-K selection (a
[B,S] GEMV + argpartition) and layout prep run on host; all GEMMs on
device.

Precision: large GEMMs use a bf16 hi/lo split (3 matmul terms, error
~2^-18 -- fp32-grade) at 1 cycle/row instead of native fp32's 4 cycles;
the small attention matmuls stay native fp32; the rank-1 "rest of V"
term (~2% of output magnitude) uses single bf16.

Attention layout: head h lives at (tile i=h//2, partition 32*(h%2)).
Scores for a head pair are ONE matmul: lhsT is a [128, 64]
block-diagonal K^T tile (two 64x32 blocks), rhs is Q^T's d-block
[128, 512], out [64, 512] -- keeps every AP base partition in {0,32,64}
(hardware restriction) and uses the full 128-lane contraction.
"""

import os
import sys
import types

import numpy as np

B, S, D, H, HD, K = 2, 2048, 1024, 16, 64, 32
NC = 8
SB = B * S // NC          # 512 query rows per core
CPB = NC // B             # cores per batch
KB = D // 128             # 128-row contraction blocks
INV_SQRT_HD = 1.0 / 8.0

_cache = {}
last_results = None


def _install_ntff_hook():
    """trace=True under axon needs antenv.axon_hooks, absent on this image.
    Synthesize it from trn_boot's ctypes implementation; degrade silently."""
    if "antenv.axon_hooks" in sys.modules:
        return
    try:
        from trn_agent_boot.trn_boot import _ntff_profile_via_ctypes

        hook = _ntff_profile_via_ctypes("/opt/axon/libaxon_pjrt.so")
        m = types.ModuleType("antenv.axon_hooks")
        m.get_axon_ntff_profile_hook = lambda: hook
        sys.modules["antenv.axon_hooks"] = m
    except Exception:
        pass


def _build():
    import concourse.bacc as bacc
    import concourse.mybir as mybir
    import concourse.tile as tile

    F32 = mybir.dt.float32
    BF16 = mybir.dt.bfloat16
    EXP = mybir.ActivationFunctionType.Exp
    CPY = mybir.ActivationFunctionType.Copy

    nc = bacc.Bacc("TRN2", target_bir_lowering=False, debug=False, num_devices=NC)

    def inp(name, shape, dt=F32):
        return nc.dram_tensor(name, shape, dt, kind="ExternalInput").ap()

    # bf16 hi/lo pairs are stacked as [2, rows, cols] (index 0=hi, 1=lo)
    xT2 = inp("xT2", [2, D, SB], BF16)      # this core's x rows, transposed, split
    wq2 = inp("wq2", [2, D, D], BF16)       # wq.T split; lhsT tile [k, m] = W[m, k]
    wv2 = inp("wv2", [2, D, D], BF16)
    wo2 = inp("wo2", [2, D, D], BF16)
    wkT = inp("wkT", [D, D])                # K path stays native fp32
    xsT32 = inp("xsT32", [D, K])            # selected x rows, transposed (fp32)
    xsr2 = inp("xsr2", [2, D, 65], BF16)    # selected rows 2x replicated + x_sum col, split
    bq8 = inp("bq8", [128, KB])             # bq[d] as [128,8]: col c = bq[128c:128c+128]
    bk8 = inp("bk8", [128, KB])
    bo8 = inp("bo8", [128, KB])
    bvr = inp("bvr", [65, D])               # bv broadcast (row 64: S*bv)
    BOc = inp("BOc", [64, 64])              # 2x block-diag 32x32 ones, fp32
    outT = nc.dram_tensor("outT", [D, SB], F32, kind="ExternalOutput").ap()

    with tile.TileContext(nc) as tc:
        with (
            tc.tile_pool(name="wqo", bufs=16) as wqo_pool,   # wq pair then wo pair
            tc.tile_pool(name="wkp", bufs=8) as wk_pool,     # wk fp32
            tc.tile_pool(name="wvp", bufs=16) as wv_pool,    # wv pair
            tc.tile_pool(name="xa", bufs=16) as xa_pool,     # xT pair, then att pair
            tc.tile_pool(name="xsr", bufs=16) as xsr_pool,
            tc.tile_pool(name="xs32", bufs=8) as xs32_pool,
            tc.tile_pool(name="qt", bufs=8) as qtp,          # Q^T resident
            tc.tile_pool(name="kbd", bufs=8) as kbdp,        # block-diag K^T
            tc.tile_pool(name="cst", bufs=1) as cst,         # consts + V tiles
            tc.tile_pool(name="sma", bufs=2) as sma,         # softmax intermediates
            tc.tile_pool(name="pq", bufs=2, space="PSUM") as pq,
            tc.tile_pool(name="pk", bufs=1, space="PSUM") as pk,
            tc.tile_pool(name="ps", bufs=3, space="PSUM") as psc,
            tc.tile_pool(name="pa", bufs=2, space="PSUM") as pa,
        ):
            # ---- resident loads ----
            def load_pair(pool, src, cols, tag):
                hi, lo = [], []
                for kb in range(KB):
                    for half, lst in ((0, hi), (1, lo)):
                        t = pool.tile([128, cols], BF16, tag=tag)
                        nc.sync.dma_start(
                            t[:], src[half, 128 * kb:128 * kb + 128, :])
                        lst.append(t)
                return hi, lo

            def const(name, src, shape, dt=F32):
                t = cst.tile(shape, dt, tag=name)
                nc.sync.dma_start(t[:], src)
                return t

            bq_sb = const("bq", bq8, [128, KB])
            bk_sb = const("bk", bk8, [128, KB])
            bo_sb = const("bo", bo8, [128, KB])
            bvr_sb = const("bvr", bvr, [65, D])
            bo_c = const("BOc", BOc, [64, 64])
            xs32 = []
            for kb in range(KB):
                t = xs32_pool.tile([128, K], F32, tag="xs32")
                nc.sync.dma_start(t[:], xsT32[128 * kb:128 * kb + 128, :])
                xs32.append(t)
            wk = []
            for kb in range(KB):
                t = wk_pool.tile([128, D], F32, tag="wk")
                nc.sync.dma_start(t[:], wkT[128 * kb:128 * kb + 128, :])
                wk.append(t)
            xh, xl = load_pair(xa_pool, xT2, SB, "xa")
            wqh, wql = load_pair(wqo_pool, wq2, D, "w")
            xsrh, xsrl = load_pair(xsr_pool, xsr2, 65, "xsr")

            def mm3(out, lh, ll, rh, rl, first, last):
                """out += lh.T@rh + lh.T@rl + ll.T@rh  (bf16 hi/lo 3-term)"""
                nc.tensor.matmul(out, lh, rh, start=first, stop=False)
                nc.tensor.matmul(out, lh, rl, start=False, stop=False)
                nc.tensor.matmul(out, ll, rh, start=False, stop=last)

            # ---- Q^T = wq @ x^T (+bq), bf16x3 ----
            QT = []
            for db in range(KB):
                ps = pq.tile([128, SB], F32, tag="pp")
                cs = slice(128 * db, 128 * db + 128)
                for kb in range(KB):
                    mm3(ps[:], wqh[kb][:, cs], wql[kb][:, cs], xh[kb][:],
                        xl[kb][:], kb == 0, kb == KB - 1)
                t = qtp.tile([128, SB], F32, tag="qt")
                nc.vector.tensor_scalar_add(t[:], ps[:], bq_sb[:, db:db + 1])
                QT.append(t)

            # ---- block-diagonal K^T tiles (native fp32 GEMM).
            # KBD[db] is [128, 64]: [0:64, 0:32] = K^T of head 2db (+bk),
            # [64:128, 32:64] = K^T of head 2db+1 (+bk), zeros elsewhere.
            KBD = []
            for db in range(KB):
                ps = pk.tile([128, K], F32, tag="pk")
                for kb in range(KB):
                    nc.tensor.matmul(ps[:], wk[kb][:, 128 * db:128 * db + 128],
                                     xs32[kb][:], start=kb == 0, stop=kb == KB - 1)
                t = kbdp.tile([128, 2 * K], F32, tag="kbd")
                nc.gpsimd.memset(t[:], 0.0)
                nc.vector.tensor_scalar_add(t[0:64, 0:K], ps[0:64, :],
                                            bk_sb[0:64, db:db + 1])
                nc.vector.tensor_scalar_add(t[64:128, K:2 * K], ps[64:128, :],
                                            bk_sb[64:128, db:db + 1])
                KBD.append(t)

            # ---- V for selected keys, 2x partition-replicated, bf16x3:
            # VR[32r+j, d] = v_sel[j, d] (+bv), r in {0,1}
            wvh, wvl = load_pair(wv_pool, wv2, D, "wv")
            VR = cst.tile([65, D], F32, tag="VR")
            for dh in range(2):
                cs = slice(512 * dh, 512 * dh + 512)
                ps = pk.tile([65, 512], F32, tag="pk")
                for kb in range(KB):
                    mm3(ps[:], xsrh[kb][:], xsrl[kb][:], wvh[kb][:, cs],
                        wvl[kb][:, cs], kb == 0, kb == KB - 1)
                nc.vector.tensor_add(VR[:, cs], ps[:], bvr_sb[:, cs])
            # row 64 of VR is V_all (the x_sum column); replicate it to
            # partitions 0 and 32 for the rank-1 matmuls
            VA = cst.tile([64, D], F32, tag="VA")
            for r in range(2):
                nc.sync.dma_start(VA[32 * r:32 * r + 1, :], VR[64:65, :])

            # ---- scores -> p-1 -> 1/denom -> weights (native fp32).
            # Tile i holds heads (2i, 2i+1): head h at partitions 32*(h%2).
            PN = []   # (p-1)/denom, fp32 [64, SB]
            IV = []   # 1/denom, fp32 (rows within a 32-group all equal)
            for i in range(KB):
                ps = psc.tile([64, SB], F32, tag="ps")
                nc.tensor.matmul(ps[:], KBD[i][:], QT[i][:], start=True, stop=True)
                p = sma.tile([64, SB], F32, tag="p")
                nc.scalar.activation(p[:], ps[:], EXP, scale=INV_SQRT_HD)
                dn = psc.tile([64, SB], F32, tag="ps")
                nc.tensor.matmul(dn[:], bo_c[:], p[:], start=True, stop=True)
                dns = sma.tile([64, SB], F32, tag="dns")
                nc.vector.tensor_scalar_add(dns[:], dn[:], float(S - K))
                iv = wv_pool.tile([64, SB], F32, tag="wv")
                nc.vector.reciprocal(iv[:], dns[:])
                pn = wv_pool.tile([64, SB], F32, tag="wv")
                nc.vector.scalar_tensor_tensor(
                    pn[:], p[:], -1.0, iv[:],
                    mybir.AluOpType.add, mybir.AluOpType.mult)
                PN.append(pn)
                IV.append(iv)

            # ---- att^T per head pair: att^T[hd, s] = V^T pn + VA/denom ----
            ATT = []  # bf16 hi/lo pairs
            for i in range(KB):  # heads (2i, 2i+1)
                ps = pa.tile([128, SB], F32, tag="pa")
                for h in (2 * i, 2 * i + 1):
                    r = 32 * (h % 2)
                    o = ps[64 * (h % 2):64 * (h % 2) + 64, :]
                    nc.tensor.matmul(
                        o, VR[r:r + 32, 64 * h:64 * h + 64],
                        PN[i][r:r + 32, :], start=True, stop=False)
                    nc.tensor.matmul(
                        o, VA[r:r + 1, 64 * h:64 * h + 64],
                        IV[i][r:r + 1, :], start=False, stop=True)
                th = xa_pool.tile([128, SB], BF16, tag="xa")
                nc.scalar.activation(th[:], ps[:], CPY)
                tl = xa_pool.tile([128, SB], BF16, tag="xa")
                nc.vector.tensor_sub(tl[:], ps[:], th[:])
                ATT.append((th, tl))

            # ---- out^T = wo @ att^T (+bo), bf16x3 ----
            woh, wol = load_pair(wqo_pool, wo2, D, "w")
            for db in range(KB):
                ps = pq.tile([128, SB], F32, tag="pp")
                cs = slice(128 * db, 128 * db + 128)
                for kb in range(KB):
                    mm3(ps[:], woh[kb][:, cs], wol[kb][:, cs], ATT[kb][0][:],
                        ATT[kb][1][:], kb == 0, kb == KB - 1)
                t = qtp.tile([128, SB], F32, tag="qt")
                nc.vector.tensor_scalar_add(t[:], ps[:], bo_sb[:, db:db + 1])
                nc.sync.dma_start(outT[128 * db:128 * db + 128, :], t[:])

    nc.compile()
    return nc


def _split(a):
    import ml_dtypes

    hi = a.astype(ml_dtypes.bfloat16)
    lo = (a - hi.astype(np.float32)).astype(ml_dtypes.bfloat16)
    return np.ascontiguousarray(np.stack([hi, lo]))


def kernel(x, wq, bq, wk, bk, wv, bv, wo, bo, ws, bs):
    global last_results
    import ml_dtypes
    from concourse.bass_utils import run_bass_kernel_spmd

    if os.environ.get("BASS_TRACE"):
        _install_ntff_hook()

    x = np.asarray(x, dtype=np.float32)
    wqT, wkT, wvT, woT = (np.ascontiguousarray(np.asarray(w, np.float32).T)
                          for w in (wq, wk, wv, wo))
    bq, bk, bv, bo = (np.asarray(b, np.float32) for b in (bq, bk, bv, bo))
    ws = np.asarray(ws, np.float32)
    bs = np.float32(bs)

    # host prep: importance -> top-K selection -> gather (GEMV + argpartition)
    imp = x.astype(np.float64) @ ws.astype(np.float64) + float(bs)   # [B,S]
    top = np.sort(np.argpartition(imp, S - K, axis=1)[:, -K:], axis=1)
    xsel = np.take_along_axis(x, top[:, :, None], axis=1)            # [B,K,D]
    xsum = x.astype(np.float64).sum(axis=1).astype(np.float32)       # [B,D]

    if "nc" not in _cache:
        _cache["nc"] = _build()
    nc = _cache["nc"]

    blockones = np.zeros((64, 64), np.float32)
    for g in range(2):
        blockones[32 * g:32 * g + 32, 32 * g:32 * g + 32] = 1.0

    def btile(b_):
        return np.ascontiguousarray(b_.reshape(KB, 128).T)

    wq2, wv2, wo2 = _split(wqT), _split(wvT), _split(woT)
    in_maps = []
    for c in range(NC):
        b = c // CPB
        r0 = (c % CPB) * SB
        xsT = np.ascontiguousarray(xsel[b].T)                        # [D, K]
        in_maps.append({
            "xT2": _split(np.ascontiguousarray(x[b, r0:r0 + SB, :].T)),
            "wq2": wq2, "wv2": wv2, "wo2": wo2, "wkT": wkT,
            "xsT32": xsT,
            "xsr2": _split(np.concatenate(
                [np.tile(xsT, (1, 2)), xsum[b][:, None]], axis=1)),
            "bq8": btile(bq), "bk8": btile(bk), "bo8": btile(bo),
            "bvr": np.ascontiguousarray(np.concatenate(
                [np.tile(bv[None, :], (64, 1)), float(S) * bv[None, :]])),
            "BOc": blockones,
        })

    res = run_bass_kernel_spmd(nc, in_maps, list(range(NC)))
    last_results = res

    out = np.empty((B, S, D), np.float32)
    for c in range(NC):
        b = c // CPB
        r0 = (c % CPB) * SB
        out[b, r0:r0 + SB, :] = res.results[c]["outT"].T
    return out


# revision 10
# speedup vs baseline: 1.2004x; 1.0475x over previous
"""DynamicSparseAttention Trainium2 kernel (8-core SPMD).

Math (exactly equivalent to the dense reference):
  The top-K mask multiplies scores by 0/1 (not -inf), so a masked key
  contributes exp(0)=1 to the softmax sum and v_j to the numerator.
  With p_j = exp(s_j/8) over the K selected keys only:
    denom = sum_top (p_j - 1) + S
    numer = sum_top (p_j - 1) * v_j + sum_all v_j
  so only the K selected K/V rows plus one "sum over all V rows" vector
  are needed -- the S x S dense attention never materializes.

Sharding: the B*S=4096 query rows are split 8 ways (cores 0-3 batch 0,
cores 4-7 batch 1); no cross-core communication. Top-K selection (a
[B,S] GEMV + argpartition) and layout prep run on host; all GEMMs on
device.

Precision: large GEMMs use a bf16 hi/lo split (3 matmul terms, error
~2^-18 -- fp32-grade) at 1 cycle/row instead of native fp32's 4 cycles;
the small attention matmuls stay native fp32; the rank-1 "rest of V"
term (~2% of output magnitude) uses single bf16.

Attention layout: head h lives at (tile i=h//2, partition 32*(h%2)).
Scores for a head pair are ONE matmul: lhsT is a [128, 64]
block-diagonal K^T tile (two 64x32 blocks), rhs is Q^T's d-block
[128, 512], out [64, 512] -- keeps every AP base partition in {0,32,64}
(hardware restriction) and uses the full 128-lane contraction.
"""

import os
import sys
import types

import numpy as np

B, S, D, H, HD, K = 2, 2048, 1024, 16, 64, 32
NC = 8
SB = B * S // NC          # 512 query rows per core
CPB = NC // B             # cores per batch
KB = D // 128             # 128-row contraction blocks
INV_SQRT_HD = 1.0 / 8.0

_cache = {}
last_results = None


def _install_ntff_hook():
    """trace=True under axon needs antenv.axon_hooks, absent on this image.
    Synthesize it from trn_boot's ctypes implementation; degrade silently."""
    if "antenv.axon_hooks" in sys.modules:
        return
    try:
        from trn_agent_boot.trn_boot import _ntff_profile_via_ctypes

        hook = _ntff_profile_via_ctypes("/opt/axon/libaxon_pjrt.so")
        m = types.ModuleType("antenv.axon_hooks")
        m.get_axon_ntff_profile_hook = lambda: hook
        sys.modules["antenv.axon_hooks"] = m
    except Exception:
        pass


def _build():
    import concourse.bacc as bacc
    import concourse.mybir as mybir
    import concourse.tile as tile

    F32 = mybir.dt.float32
    BF16 = mybir.dt.bfloat16
    EXP = mybir.ActivationFunctionType.Exp
    CPY = mybir.ActivationFunctionType.Copy

    nc = bacc.Bacc("TRN2", target_bir_lowering=False, debug=False, num_devices=NC)

    def inp(name, shape, dt=F32):
        return nc.dram_tensor(name, shape, dt, kind="ExternalInput").ap()

    # bf16 hi/lo pairs are stacked as [2, rows, cols] (index 0=hi, 1=lo)
    xT2 = inp("xT2", [2, D, SB], BF16)      # this core's x rows, transposed, split
    wq2 = inp("wq2", [2, D, D], BF16)       # wq.T split; lhsT tile [k, m] = W[m, k]
    wv2 = inp("wv2", [2, D, D], BF16)
    wo2 = inp("wo2", [2, D, D], BF16)
    wk2 = inp("wk2", [2, D, D], BF16)
    xs322 = inp("xs322", [2, D, K], BF16)   # selected x rows, transposed, split
    xsr2 = inp("xsr2", [2, D, 65], BF16)    # selected rows 2x replicated + x_sum col, split
    bq8 = inp("bq8", [128, KB])             # bq[d] as [128,8]: col c = bq[128c:128c+128]
    bk8 = inp("bk8", [128, KB])
    bo8 = inp("bo8", [128, KB])
    bvr = inp("bvr", [65, D])               # bv broadcast (row 64: S*bv)
    BOc = inp("BOc", [64, 64], BF16)        # 2x block-diag 32x32 ones (exact)
    ident = inp("ident", [K, K])            # fp32 identity for PE transpose
    outT = nc.dram_tensor("outT", [D, SB], F32, kind="ExternalOutput").ap()

    with tile.TileContext(nc) as tc:
        with (
            tc.tile_pool(name="wqo", bufs=16) as wqo_pool,   # wq pair then wo pair
            tc.tile_pool(name="wkp", bufs=16) as wk_pool,     # wk fp32
            tc.tile_pool(name="wvp", bufs=16) as wv_pool,    # wv pair
            tc.tile_pool(name="xa", bufs=16) as xa_pool,     # xT pair, then att pair
            tc.tile_pool(name="xsr", bufs=16) as xsr_pool,
            tc.tile_pool(name="xs32", bufs=16) as xs32_pool,
            tc.tile_pool(name="qt", bufs=8) as qtp,          # Q^T resident
            tc.tile_pool(name="kbd", bufs=8) as kbdp,        # block-diag K^T
            tc.tile_pool(name="cst", bufs=1) as cst,         # consts + V tiles
            tc.tile_pool(name="sma", bufs=2) as sma,         # softmax intermediates
            tc.tile_pool(name="pq", bufs=2, space="PSUM") as pq,
            tc.tile_pool(name="pk", bufs=1, space="PSUM") as pk,
            tc.tile_pool(name="ps", bufs=3, space="PSUM") as psc,
            tc.tile_pool(name="pa", bufs=2, space="PSUM") as pa,
        ):
            # ---- resident loads ----
            def load_pair(pool, src, cols, tag):
                hi, lo = [], []
                for kb in range(KB):
                    for half, lst in ((0, hi), (1, lo)):
                        t = pool.tile([128, cols], BF16, tag=tag)
                        nc.sync.dma_start(
                            t[:], src[half, 128 * kb:128 * kb + 128, :])
                        lst.append(t)
                return hi, lo

            def const(name, src, shape, dt=F32):
                t = cst.tile(shape, dt, tag=name)
                nc.sync.dma_start(t[:], src)
                return t

            bq_sb = const("bq", bq8, [128, KB])
            bk_sb = const("bk", bk8, [128, KB])
            bo_sb = const("bo", bo8, [128, KB])
            bvr_sb = const("bvr", bvr, [65, D])
            bo_c = const("BOc", BOc, [64, 64], BF16)
            id_sb = const("ident", ident, [K, K])
            xh, xl = load_pair(xa_pool, xT2, SB, "xa")
            wqh, wql = load_pair(wqo_pool, wq2, D, "w")
            xs32h, xs32l = load_pair(xs32_pool, xs322, K, "xs32")
            wkh, wkl = load_pair(wk_pool, wk2, D, "wk")
            xsrh, xsrl = load_pair(xsr_pool, xsr2, 65, "xsr")

            def mm3(out, lh, ll, rh, rl, first, last):
                """out += lh.T@rh + lh.T@rl + ll.T@rh  (bf16 hi/lo 3-term)"""
                nc.tensor.matmul(out, lh, rh, start=first, stop=False)
                nc.tensor.matmul(out, lh, rl, start=False, stop=False)
                nc.tensor.matmul(out, ll, rh, start=False, stop=last)

            # ---- Q^T = wq @ x^T (+bq), bf16x3 ----
            QT = []
            for db in range(KB):
                ps = pq.tile([128, SB], F32, tag="pp")
                cs = slice(128 * db, 128 * db + 128)
                for kb in range(KB):
                    mm3(ps[:], wqh[kb][:, cs], wql[kb][:, cs], xh[kb][:],
                        xl[kb][:], kb == 0, kb == KB - 1)
                t = qtp.tile([128, SB], F32, tag="qt")
                nc.vector.tensor_scalar_add(t[:], ps[:], bq_sb[:, db:db + 1])
                QT.append(t)

            # ---- K for selected keys, bf16x3 in [j, d] layout, then PE
            # transposes build block-diagonal K^T tiles: KBD[db] is [128, 64]
            # with [0:64, 0:32] = K^T of head 2db (+bk), [64:128, 32:64] =
            # K^T of head 2db+1 (+bk), zeros elsewhere.
            Ksb = cst.tile([K, D], F32, tag="Ksb")
            for dh in range(2):
                cs = slice(512 * dh, 512 * dh + 512)
                ps = pk.tile([K, 512], F32, tag="pk")
                for kb in range(KB):
                    mm3(ps[:], xs32h[kb][:], xs32l[kb][:], wkh[kb][:, cs],
                        wkl[kb][:, cs], kb == 0, kb == KB - 1)
                nc.vector.tensor_copy(Ksb[:, cs], ps[:])
            KBD = []
            for db in range(KB):
                pt = pk.tile([128, K], F32, tag="pk")
                nc.tensor.transpose(pt[:], Ksb[:, 128 * db:128 * db + 128],
                                    id_sb[:])
                t = kbdp.tile([128, 2 * K], F32, tag="kbd")
                nc.gpsimd.memset(t[:], 0.0)
                nc.vector.tensor_scalar_add(t[0:64, 0:K], pt[0:64, :],
                                            bk_sb[0:64, db:db + 1])
                nc.vector.tensor_scalar_add(t[64:128, K:2 * K], pt[64:128, :],
                                            bk_sb[64:128, db:db + 1])
                KBD.append(t)

            # ---- V for selected keys, 2x partition-replicated, bf16x3:
            # VR[32r+j, d] = v_sel[j, d] (+bv), r in {0,1}
            wvh, wvl = load_pair(wv_pool, wv2, D, "wv")
            VR = cst.tile([65, D], F32, tag="VR")
            for dh in range(2):
                cs = slice(512 * dh, 512 * dh + 512)
                ps = pk.tile([65, 512], F32, tag="pk")
                for kb in range(KB):
                    mm3(ps[:], xsrh[kb][:], xsrl[kb][:], wvh[kb][:, cs],
                        wvl[kb][:, cs], kb == 0, kb == KB - 1)
                nc.vector.tensor_add(VR[:, cs], ps[:], bvr_sb[:, cs])
            # row 64 of VR is V_all (the x_sum column); replicate it to
            # partitions 0 and 32 for the rank-1 matmuls
            VA = cst.tile([64, D], F32, tag="VA")
            for r in range(2):
                nc.sync.dma_start(VA[32 * r:32 * r + 1, :], VR[64:65, :])

            # ---- scores -> p-1 -> 1/denom -> weights (native fp32).
            # Tile i holds heads (2i, 2i+1): head h at partitions 32*(h%2).
            PN = []   # (p-1)/denom, fp32 [64, SB]
            IV = []   # 1/denom, fp32 (rows within a 32-group all equal)
            for i in range(KB):
                ps = psc.tile([64, SB], F32, tag="ps")
                nc.tensor.matmul(ps[:], KBD[i][:], QT[i][:], start=True, stop=True)
                p = sma.tile([64, SB], F32, tag="p")
                nc.scalar.activation(p[:], ps[:], EXP, scale=INV_SQRT_HD)
                ph = sma.tile([64, SB], BF16, tag="ph")
                nc.scalar.activation(ph[:], ps[:], EXP, scale=INV_SQRT_HD)
                pl = sma.tile([64, SB], BF16, tag="pl")
                nc.vector.tensor_sub(pl[:], p[:], ph[:])
                dn = psc.tile([64, SB], F32, tag="ps")
                nc.tensor.matmul(dn[:], bo_c[:], ph[:], start=True, stop=False)
                nc.tensor.matmul(dn[:], bo_c[:], pl[:], start=False, stop=True)
                dns = sma.tile([64, SB], F32, tag="dns")
                nc.vector.tensor_scalar_add(dns[:], dn[:], float(S - K))
                iv = wv_pool.tile([64, SB], F32, tag="wv")
                nc.vector.reciprocal(iv[:], dns[:])
                pn = wv_pool.tile([64, SB], F32, tag="wv")
                nc.vector.scalar_tensor_tensor(
                    pn[:], p[:], -1.0, iv[:],
                    mybir.AluOpType.add, mybir.AluOpType.mult)
                PN.append(pn)
                IV.append(iv)

            # ---- att^T per head pair: att^T[hd, s] = V^T pn + VA/denom ----
            ATT = []  # bf16 hi/lo pairs
            for i in range(KB):  # heads (2i, 2i+1)
                ps = pa.tile([128, SB], F32, tag="pa")
                for h in (2 * i, 2 * i + 1):
                    r = 32 * (h % 2)
                    o = ps[64 * (h % 2):64 * (h % 2) + 64, :]
                    nc.tensor.matmul(
                        o, VR[r:r + 32, 64 * h:64 * h + 64],
                        PN[i][r:r + 32, :], start=True, stop=False)
                    nc.tensor.matmul(
                        o, VA[r:r + 1, 64 * h:64 * h + 64],
                        IV[i][r:r + 1, :], start=False, stop=True)
                th = xa_pool.tile([128, SB], BF16, tag="xa")
                nc.scalar.activation(th[:], ps[:], CPY)
                tl = xa_pool.tile([128, SB], BF16, tag="xa")
                nc.vector.tensor_sub(tl[:], ps[:], th[:])
                ATT.append((th, tl))

            # ---- out^T = wo @ att^T (+bo), bf16x3 ----
            woh, wol = load_pair(wqo_pool, wo2, D, "w")
            for db in range(KB):
                ps = pq.tile([128, SB], F32, tag="pp")
                cs = slice(128 * db, 128 * db + 128)
                for kb in range(KB):
                    mm3(ps[:], woh[kb][:, cs], wol[kb][:, cs], ATT[kb][0][:],
                        ATT[kb][1][:], kb == 0, kb == KB - 1)
                t = qtp.tile([128, SB], F32, tag="qt")
                nc.vector.tensor_scalar_add(t[:], ps[:], bo_sb[:, db:db + 1])
                nc.sync.dma_start(outT[128 * db:128 * db + 128, :], t[:])

    nc.compile()
    return nc


def _split(a):
    import ml_dtypes

    hi = a.astype(ml_dtypes.bfloat16)
    lo = (a - hi.astype(np.float32)).astype(ml_dtypes.bfloat16)
    return np.ascontiguousarray(np.stack([hi, lo]))


def kernel(x, wq, bq, wk, bk, wv, bv, wo, bo, ws, bs):
    global last_results
    import ml_dtypes
    from concourse.bass_utils import run_bass_kernel_spmd

    if os.environ.get("BASS_TRACE"):
        _install_ntff_hook()

    x = np.asarray(x, dtype=np.float32)
    wqT, wkT, wvT, woT = (np.ascontiguousarray(np.asarray(w, np.float32).T)
                          for w in (wq, wk, wv, wo))
    bq, bk, bv, bo = (np.asarray(b, np.float32) for b in (bq, bk, bv, bo))
    ws = np.asarray(ws, np.float32)
    bs = np.float32(bs)

    # host prep: importance -> top-K selection -> gather (GEMV + argpartition)
    imp = x.astype(np.float64) @ ws.astype(np.float64) + float(bs)   # [B,S]
    top = np.sort(np.argpartition(imp, S - K, axis=1)[:, -K:], axis=1)
    xsel = np.take_along_axis(x, top[:, :, None], axis=1)            # [B,K,D]
    xsum = x.astype(np.float64).sum(axis=1).astype(np.float32)       # [B,D]

    if "nc" not in _cache:
        _cache["nc"] = _build()
    nc = _cache["nc"]

    blockones = np.zeros((64, 64), np.float32)
    for g in range(2):
        blockones[32 * g:32 * g + 32, 32 * g:32 * g + 32] = 1.0

    def btile(b_):
        return np.ascontiguousarray(b_.reshape(KB, 128).T)

    wq2, wv2, wo2, wk2 = _split(wqT), _split(wvT), _split(woT), _split(wkT)
    in_maps = []
    for c in range(NC):
        b = c // CPB
        r0 = (c % CPB) * SB
        xsT = np.ascontiguousarray(xsel[b].T)                        # [D, K]
        in_maps.append({
            "xT2": _split(np.ascontiguousarray(x[b, r0:r0 + SB, :].T)),
            "wq2": wq2, "wv2": wv2, "wo2": wo2, "wk2": wk2,
            "xs322": _split(xsT),
            "xsr2": _split(np.concatenate(
                [np.tile(xsT, (1, 2)), xsum[b][:, None]], axis=1)),
            "bq8": btile(bq), "bk8": btile(bk), "bo8": btile(bo),
            "bvr": np.ascontiguousarray(np.concatenate(
                [np.tile(bv[None, :], (64, 1)), float(S) * bv[None, :]])),
            "BOc": blockones.astype(ml_dtypes.bfloat16),
            "ident": np.eye(K, dtype=np.float32),
        })

    res = run_bass_kernel_spmd(nc, in_maps, list(range(NC)))
    last_results = res

    out = np.empty((B, S, D), np.float32)
    for c in range(NC):
        b = c // CPB
        r0 = (c % CPB) * SB
        out[b, r0:r0 + SB, :] = res.results[c]["outT"].T
    return out


# revision 12
# speedup vs baseline: 1.2096x; 1.0076x over previous
"""DynamicSparseAttention Trainium2 kernel (8-core SPMD).

Math (exactly equivalent to the dense reference):
  The top-K mask multiplies scores by 0/1 (not -inf), so a masked key
  contributes exp(0)=1 to the softmax sum and v_j to the numerator.
  With p_j = exp(s_j/8) over the K selected keys only:
    denom = sum_top (p_j - 1) + S
    numer = sum_top (p_j - 1) * v_j + sum_all v_j
  so only the K selected K/V rows plus one "sum over all V rows" vector
  are needed -- the S x S dense attention never materializes.

Sharding: the B*S=4096 query rows are split 8 ways (cores 0-3 batch 0,
cores 4-7 batch 1); no cross-core communication. Top-K selection (a
[B,S] GEMV + argpartition) and layout prep run on host; all GEMMs on
device.

Precision: large GEMMs use a bf16 hi/lo split (3 matmul terms, error
~2^-18 -- fp32-grade) at 1 cycle/row instead of native fp32's 4 cycles;
the small attention matmuls stay native fp32; the rank-1 "rest of V"
term (~2% of output magnitude) uses single bf16.

Attention layout: head h lives at (tile i=h//2, partition 32*(h%2)).
Scores for a head pair are ONE matmul: lhsT is a [128, 64]
block-diagonal K^T tile (two 64x32 blocks), rhs is Q^T's d-block
[128, 512], out [64, 512] -- keeps every AP base partition in {0,32,64}
(hardware restriction) and uses the full 128-lane contraction.
"""

import os
import sys
import types

import numpy as np

B, S, D, H, HD, K = 2, 2048, 1024, 16, 64, 32
NC = 8
SB = B * S // NC          # 512 query rows per core
CPB = NC // B             # cores per batch
KB = D // 128             # 128-row contraction blocks
INV_SQRT_HD = 1.0 / 8.0

_cache = {}
last_results = None


def _install_ntff_hook():
    """trace=True under axon needs antenv.axon_hooks, absent on this image.
    Synthesize it from trn_boot's ctypes implementation; degrade silently."""
    if "antenv.axon_hooks" in sys.modules:
        return
    try:
        from trn_agent_boot.trn_boot import _ntff_profile_via_ctypes

        hook = _ntff_profile_via_ctypes("/opt/axon/libaxon_pjrt.so")
        m = types.ModuleType("antenv.axon_hooks")
        m.get_axon_ntff_profile_hook = lambda: hook
        sys.modules["antenv.axon_hooks"] = m
    except Exception:
        pass


def _build():
    import concourse.bacc as bacc
    import concourse.mybir as mybir
    import concourse.tile as tile

    F32 = mybir.dt.float32
    BF16 = mybir.dt.bfloat16
    EXP = mybir.ActivationFunctionType.Exp
    CPY = mybir.ActivationFunctionType.Copy
    IDN = mybir.ActivationFunctionType.Identity

    nc = bacc.Bacc("TRN2", target_bir_lowering=False, debug=False, num_devices=NC)

    def inp(name, shape, dt=F32):
        return nc.dram_tensor(name, shape, dt, kind="ExternalInput").ap()

    # bf16 hi/lo pairs are stacked as [2, rows, cols] (index 0=hi, 1=lo)
    xT2 = inp("xT2", [2, D, SB], BF16)      # this core's x rows, transposed, split
    wq2 = inp("wq2", [2, D, D], BF16)       # wq.T split; lhsT tile [k, m] = W[m, k]
    wv2 = inp("wv2", [2, D, D], BF16)
    wo2 = inp("wo2", [2, D, D], BF16)
    wk2 = inp("wk2", [2, D, D], BF16)
    xs322 = inp("xs322", [2, D, K], BF16)   # selected x rows, transposed, split
    xsr2 = inp("xsr2", [2, D, 65], BF16)    # selected rows 2x replicated + x_sum col, split
    bq8 = inp("bq8", [128, KB])             # bq[d] as [128,8]: col c = bq[128c:128c+128]
    bk8 = inp("bk8", [128, KB])
    bo8 = inp("bo8", [128, KB])
    bvr = inp("bvr", [65, D])               # bv broadcast (row 64: S*bv)
    BOc = inp("BOc", [64, 64], BF16)        # 2x block-diag 32x32 ones (exact)
    ident = inp("ident", [K, K])            # fp32 identity for PE transpose
    smk = inp("smk", [64, 1])               # S-K softmax denominator constant
    outT = nc.dram_tensor("outT", [D, SB], F32, kind="ExternalOutput").ap()

    with tile.TileContext(nc) as tc:
        with (
            tc.tile_pool(name="wqo", bufs=16) as wqo_pool,   # wq pair then wo pair
            tc.tile_pool(name="wkp", bufs=16) as wk_pool,     # wk fp32
            tc.tile_pool(name="wvp", bufs=16) as wv_pool,    # wv pair
            tc.tile_pool(name="xa", bufs=16) as xa_pool,     # xT pair, then att pair
            tc.tile_pool(name="xsr", bufs=16) as xsr_pool,
            tc.tile_pool(name="xs32", bufs=16) as xs32_pool,
            tc.tile_pool(name="qt", bufs=8) as qtp,          # Q^T resident
            tc.tile_pool(name="kbd", bufs=8) as kbdp,        # block-diag K^T
            tc.tile_pool(name="cst", bufs=1) as cst,         # consts + V tiles
            tc.tile_pool(name="sma", bufs=4) as sma,         # softmax intermediates
            tc.tile_pool(name="pq", bufs=2, space="PSUM") as pq,
            tc.tile_pool(name="pk", bufs=1, space="PSUM") as pk,
            tc.tile_pool(name="ps", bufs=3, space="PSUM") as psc,
            tc.tile_pool(name="pa", bufs=2, space="PSUM") as pa,
        ):
            # ---- resident loads ----
            def load_pair(pool, src, cols, tag):
                hi, lo = [], []
                for kb in range(KB):
                    for half, lst in ((0, hi), (1, lo)):
                        t = pool.tile([128, cols], BF16, tag=tag)
                        nc.sync.dma_start(
                            t[:], src[half, 128 * kb:128 * kb + 128, :])
                        lst.append(t)
                return hi, lo

            def const(name, src, shape, dt=F32):
                t = cst.tile(shape, dt, tag=name)
                nc.sync.dma_start(t[:], src)
                return t

            bq_sb = const("bq", bq8, [128, KB])
            bk_sb = const("bk", bk8, [128, KB])
            bo_sb = const("bo", bo8, [128, KB])
            bvr_sb = const("bvr", bvr, [65, D])
            bo_c = const("BOc", BOc, [64, 64], BF16)
            id_sb = const("ident", ident, [K, K])
            smk_sb = const("smk", smk, [64, 1])
            xh, xl = load_pair(xa_pool, xT2, SB, "xa")
            wqh, wql = load_pair(wqo_pool, wq2, D, "w")
            xs32h, xs32l = load_pair(xs32_pool, xs322, K, "xs32")
            wkh, wkl = load_pair(wk_pool, wk2, D, "wk")
            xsrh, xsrl = load_pair(xsr_pool, xsr2, 65, "xsr")

            def mm3(out, lh, ll, rh, rl, first, last):
                """out += lh.T@rh + lh.T@rl + ll.T@rh  (bf16 hi/lo 3-term)"""
                nc.tensor.matmul(out, lh, rh, start=first, stop=False)
                nc.tensor.matmul(out, lh, rl, start=False, stop=False)
                nc.tensor.matmul(out, ll, rh, start=False, stop=last)

            # ---- Q^T = wq @ x^T (+bq), bf16x3 ----
            QT = []
            for db in range(KB):
                ps = pq.tile([128, SB], F32, tag="pp")
                cs = slice(128 * db, 128 * db + 128)
                for kb in range(KB):
                    mm3(ps[:], wqh[kb][:, cs], wql[kb][:, cs], xh[kb][:],
                        xl[kb][:], kb == 0, kb == KB - 1)
                t = qtp.tile([128, SB], F32, tag="qt")
                nc.vector.tensor_scalar_add(t[:], ps[:], bq_sb[:, db:db + 1])
                QT.append(t)

            # ---- K for selected keys, bf16x3 in [j, d] layout, then PE
            # transposes build block-diagonal K^T tiles: KBD[db] is [128, 64]
            # with [0:64, 0:32] = K^T of head 2db (+bk), [64:128, 32:64] =
            # K^T of head 2db+1 (+bk), zeros elsewhere.
            Ksb = cst.tile([K, D], F32, tag="Ksb")
            for dh in range(2):
                cs = slice(512 * dh, 512 * dh + 512)
                ps = pk.tile([K, 512], F32, tag="pk")
                for kb in range(KB):
                    mm3(ps[:], xs32h[kb][:], xs32l[kb][:], wkh[kb][:, cs],
                        wkl[kb][:, cs], kb == 0, kb == KB - 1)
                nc.scalar.activation(Ksb[:, cs], ps[:], IDN)
            KBD = []
            for db in range(KB):
                pt = pk.tile([128, K], F32, tag="pk")
                nc.tensor.transpose(pt[:], Ksb[:, 128 * db:128 * db + 128],
                                    id_sb[:])
                t = kbdp.tile([128, 2 * K], F32, tag="kbd")
                nc.gpsimd.memset(t[:], 0.0)
                nc.vector.tensor_scalar_add(t[0:64, 0:K], pt[0:64, :],
                                            bk_sb[0:64, db:db + 1])
                nc.vector.tensor_scalar_add(t[64:128, K:2 * K], pt[64:128, :],
                                            bk_sb[64:128, db:db + 1])
                KBD.append(t)

            # ---- V for selected keys, 2x partition-replicated, bf16x3:
            # VR[32r+j, d] = v_sel[j, d] (+bv), r in {0,1}
            wvh, wvl = load_pair(wv_pool, wv2, D, "wv")
            VR = cst.tile([65, D], F32, tag="VR")
            for dh in range(2):
                cs = slice(512 * dh, 512 * dh + 512)
                ps = pk.tile([65, 512], F32, tag="pk")
                for kb in range(KB):
                    mm3(ps[:], xsrh[kb][:], xsrl[kb][:], wvh[kb][:, cs],
                        wvl[kb][:, cs], kb == 0, kb == KB - 1)
                nc.vector.tensor_add(VR[:, cs], ps[:], bvr_sb[:, cs])
            # row 64 of VR is V_all (the x_sum column); replicate it to
            # partitions 0 and 32 for the rank-1 matmuls
            VA = cst.tile([64, D], F32, tag="VA")
            for r in range(2):
                nc.sync.dma_start(VA[32 * r:32 * r + 1, :], VR[64:65, :])

            # ---- scores -> p-1 -> 1/denom -> weights (native fp32).
            # Tile i holds heads (2i, 2i+1): head h at partitions 32*(h%2).
            PN = []   # (p-1)/denom, fp32 [64, SB]
            IV = []   # 1/denom, fp32 (rows within a 32-group all equal)
            for i in range(KB):
                ps = psc.tile([64, SB], F32, tag="ps")
                nc.tensor.matmul(ps[:], KBD[i][:], QT[i][:], start=True, stop=True)
                p = sma.tile([64, SB], F32, tag="p")
                nc.scalar.activation(p[:], ps[:], EXP, scale=INV_SQRT_HD)
                ph = sma.tile([64, SB], BF16, tag="ph")
                nc.scalar.activation(ph[:], ps[:], EXP, scale=INV_SQRT_HD)
                pl = sma.tile([64, SB], BF16, tag="pl")
                nc.vector.tensor_sub(pl[:], p[:], ph[:])
                dn = psc.tile([64, SB], F32, tag="ps")
                nc.tensor.matmul(dn[:], bo_c[:], ph[:], start=True, stop=False)
                nc.tensor.matmul(dn[:], bo_c[:], pl[:], start=False, stop=True)
                dns = sma.tile([64, SB], F32, tag="dns")
                nc.scalar.activation(dns[:], dn[:], IDN, bias=smk_sb[:])
                iv = wv_pool.tile([64, SB], F32, tag="wv")
                nc.vector.reciprocal(iv[:], dns[:])
                pn = wv_pool.tile([64, SB], F32, tag="wv")
                nc.vector.scalar_tensor_tensor(
                    pn[:], p[:], -1.0, iv[:],
                    mybir.AluOpType.add, mybir.AluOpType.mult)
                PN.append(pn)
                IV.append(iv)

            # ---- att^T per head pair: att^T[hd, s] = V^T pn + VA/denom ----
            ATT = []  # bf16 hi/lo pairs
            for i in range(KB):  # heads (2i, 2i+1)
                ps = pa.tile([128, SB], F32, tag="pa")
                for h in (2 * i, 2 * i + 1):
                    r = 32 * (h % 2)
                    o = ps[64 * (h % 2):64 * (h % 2) + 64, :]
                    nc.tensor.matmul(
                        o, VR[r:r + 32, 64 * h:64 * h + 64],
                        PN[i][r:r + 32, :], start=True, stop=False)
                    nc.tensor.matmul(
                        o, VA[r:r + 1, 64 * h:64 * h + 64],
                        IV[i][r:r + 1, :], start=False, stop=True)
                th = xa_pool.tile([128, SB], BF16, tag="xa")
                nc.scalar.activation(th[:], ps[:], CPY)
                tl = xa_pool.tile([128, SB], BF16, tag="xa")
                nc.vector.tensor_sub(tl[:], ps[:], th[:])
                ATT.append((th, tl))

            # ---- out^T = wo @ att^T (+bo), bf16x3 ----
            woh, wol = load_pair(wqo_pool, wo2, D, "w")
            for db in range(KB):
                ps = pq.tile([128, SB], F32, tag="pp")
                cs = slice(128 * db, 128 * db + 128)
                for kb in range(KB):
                    mm3(ps[:], woh[kb][:, cs], wol[kb][:, cs], ATT[kb][0][:],
                        ATT[kb][1][:], kb == 0, kb == KB - 1)
                t = qtp.tile([128, SB], F32, tag="qt")
                nc.vector.tensor_scalar_add(t[:], ps[:], bo_sb[:, db:db + 1])
                nc.sync.dma_start(outT[128 * db:128 * db + 128, :], t[:])

    nc.compile()
    return nc


def _split(a):
    import ml_dtypes

    hi = a.astype(ml_dtypes.bfloat16)
    lo = (a - hi.astype(np.float32)).astype(ml_dtypes.bfloat16)
    return np.ascontiguousarray(np.stack([hi, lo]))


def kernel(x, wq, bq, wk, bk, wv, bv, wo, bo, ws, bs):
    global last_results
    import ml_dtypes
    from concourse.bass_utils import run_bass_kernel_spmd

    if os.environ.get("BASS_TRACE"):
        _install_ntff_hook()

    x = np.asarray(x, dtype=np.float32)
    wqT, wkT, wvT, woT = (np.ascontiguousarray(np.asarray(w, np.float32).T)
                          for w in (wq, wk, wv, wo))
    bq, bk, bv, bo = (np.asarray(b, np.float32) for b in (bq, bk, bv, bo))
    ws = np.asarray(ws, np.float32)
    bs = np.float32(bs)

    # host prep: importance -> top-K selection -> gather (GEMV + argpartition)
    imp = x.astype(np.float64) @ ws.astype(np.float64) + float(bs)   # [B,S]
    top = np.sort(np.argpartition(imp, S - K, axis=1)[:, -K:], axis=1)
    xsel = np.take_along_axis(x, top[:, :, None], axis=1)            # [B,K,D]
    xsum = x.astype(np.float64).sum(axis=1).astype(np.float32)       # [B,D]

    if "nc" not in _cache:
        _cache["nc"] = _build()
    nc = _cache["nc"]

    blockones = np.zeros((64, 64), np.float32)
    for g in range(2):
        blockones[32 * g:32 * g + 32, 32 * g:32 * g + 32] = 1.0

    def btile(b_):
        return np.ascontiguousarray(b_.reshape(KB, 128).T)

    wq2, wv2, wo2, wk2 = _split(wqT), _split(wvT), _split(woT), _split(wkT)
    in_maps = []
    for c in range(NC):
        b = c // CPB
        r0 = (c % CPB) * SB
        xsT = np.ascontiguousarray(xsel[b].T)                        # [D, K]
        in_maps.append({
            "xT2": _split(np.ascontiguousarray(x[b, r0:r0 + SB, :].T)),
            "wq2": wq2, "wv2": wv2, "wo2": wo2, "wk2": wk2,
            "xs322": _split(xsT),
            "xsr2": _split(np.concatenate(
                [np.tile(xsT, (1, 2)), xsum[b][:, None]], axis=1)),
            "bq8": btile(bq), "bk8": btile(bk), "bo8": btile(bo),
            "bvr": np.ascontiguousarray(np.concatenate(
                [np.tile(bv[None, :], (64, 1)), float(S) * bv[None, :]])),
            "BOc": blockones.astype(ml_dtypes.bfloat16),
            "ident": np.eye(K, dtype=np.float32),
            "smk": np.full((64, 1), float(S - K), np.float32),
        })

    res = run_bass_kernel_spmd(nc, in_maps, list(range(NC)))
    last_results = res

    out = np.empty((B, S, D), np.float32)
    for c in range(NC):
        b = c // CPB
        r0 = (c % CPB) * SB
        out[b, r0:r0 + SB, :] = res.results[c]["outT"].T
    return out


# revision 13
# speedup vs baseline: 1.2407x; 1.0257x over previous
"""DynamicSparseAttention Trainium2 kernel (8-core SPMD).

Math (exactly equivalent to the dense reference):
  The top-K mask multiplies scores by 0/1 (not -inf), so a masked key
  contributes exp(0)=1 to the softmax sum and v_j to the numerator.
  With p_j = exp(s_j/8) over the K selected keys only:
    denom = sum_top (p_j - 1) + S
    numer = sum_top (p_j - 1) * v_j + sum_all v_j
  so only the K selected K/V rows plus one "sum over all V rows" vector
  are needed -- the S x S dense attention never materializes.

Sharding: the B*S=4096 query rows are split 8 ways (cores 0-3 batch 0,
cores 4-7 batch 1); no cross-core communication. Top-K selection (a
[B,S] GEMV + argpartition) and layout prep run on host; all GEMMs on
device.

Precision: large GEMMs use a bf16 hi/lo split (3 matmul terms, error
~2^-18 -- fp32-grade) at 1 cycle/row instead of native fp32's 4 cycles;
the small attention matmuls stay native fp32; the rank-1 "rest of V"
term (~2% of output magnitude) uses single bf16.

Attention layout: head h lives at (tile i=h//2, partition 32*(h%2)).
Scores for a head pair are ONE matmul: lhsT is a [128, 64]
block-diagonal K^T tile (two 64x32 blocks), rhs is Q^T's d-block
[128, 512], out [64, 512] -- keeps every AP base partition in {0,32,64}
(hardware restriction) and uses the full 128-lane contraction.
"""

import os
import sys
import types

import numpy as np

B, S, D, H, HD, K = 2, 2048, 1024, 16, 64, 32
NC = 8
SB = B * S // NC          # 512 query rows per core
CPB = NC // B             # cores per batch
KB = D // 128             # 128-row contraction blocks
INV_SQRT_HD = 1.0 / 8.0

_cache = {}
last_results = None


def _install_ntff_hook():
    """trace=True under axon needs antenv.axon_hooks, absent on this image.
    Synthesize it from trn_boot's ctypes implementation; degrade silently."""
    if "antenv.axon_hooks" in sys.modules:
        return
    try:
        from trn_agent_boot.trn_boot import _ntff_profile_via_ctypes

        hook = _ntff_profile_via_ctypes("/opt/axon/libaxon_pjrt.so")
        m = types.ModuleType("antenv.axon_hooks")
        m.get_axon_ntff_profile_hook = lambda: hook
        sys.modules["antenv.axon_hooks"] = m
    except Exception:
        pass


def _build():
    import concourse.bacc as bacc
    import concourse.mybir as mybir
    import concourse.tile as tile

    F32 = mybir.dt.float32
    BF16 = mybir.dt.bfloat16
    EXP = mybir.ActivationFunctionType.Exp
    CPY = mybir.ActivationFunctionType.Copy
    IDN = mybir.ActivationFunctionType.Identity

    nc = bacc.Bacc("TRN2", target_bir_lowering=False, debug=False, num_devices=NC)

    def inp(name, shape, dt=F32):
        return nc.dram_tensor(name, shape, dt, kind="ExternalInput").ap()

    # bf16 hi/lo pairs are stacked as [2, rows, cols] (index 0=hi, 1=lo)
    xT2 = inp("xT2", [2, D, SB], BF16)      # this core's x rows, transposed, split
    wq2 = inp("wq2", [2, D, D], BF16)       # wq.T split; lhsT tile [k, m] = W[m, k]
    wv2 = inp("wv2", [2, D, D], BF16)
    wo2 = inp("wo2", [2, D, D], BF16)
    wk2 = inp("wk2", [2, D, D], BF16)
    xs322 = inp("xs322", [2, D, K], BF16)   # selected x rows, transposed, split
    xsr2 = inp("xsr2", [2, D, 65], BF16)    # selected rows 2x replicated + x_sum col, split
    bq8 = inp("bq8", [128, KB])             # bq[d] as [128,8]: col c = bq[128c:128c+128]
    bk8 = inp("bk8", [128, KB])
    bo8 = inp("bo8", [128, KB])
    bvr = inp("bvr", [65, D])               # bv broadcast (row 64: S*bv)
    BOc = inp("BOc", [64, 64], BF16)        # 2x block-diag 32x32 ones (exact)
    ident = inp("ident", [K, K])            # fp32 identity for PE transpose
    smk = inp("smk", [64, 1])               # S-K softmax denominator constant
    outT = nc.dram_tensor("outT", [D, SB], F32, kind="ExternalOutput").ap()

    with tile.TileContext(nc) as tc:
        with (
            tc.tile_pool(name="wqo", bufs=16) as wqo_pool,   # wq pair then wo pair
            tc.tile_pool(name="wkp", bufs=16) as wk_pool,     # wk fp32
            tc.tile_pool(name="wvp", bufs=16) as wv_pool,    # wv pair
            tc.tile_pool(name="xa", bufs=16) as xa_pool,     # xT pair, then att pair
            tc.tile_pool(name="xsr", bufs=16) as xsr_pool,
            tc.tile_pool(name="xs32", bufs=16) as xs32_pool,
            tc.tile_pool(name="qt", bufs=8) as qtp,          # Q^T resident
            tc.tile_pool(name="kbd", bufs=8) as kbdp,        # block-diag K^T
            tc.tile_pool(name="cst", bufs=1) as cst,         # consts + V tiles
            tc.tile_pool(name="sma", bufs=4) as sma,         # softmax intermediates
            tc.tile_pool(name="pq", bufs=2, space="PSUM") as pq,
            tc.tile_pool(name="ps", bufs=4, space="PSUM") as psc,
            tc.tile_pool(name="pa", bufs=2, space="PSUM") as pa,
        ):
            # ---- resident loads ----
            def load_pair(pool, src, cols, tag):
                hi, lo = [], []
                for kb in range(KB):
                    for half, lst in ((0, hi), (1, lo)):
                        t = pool.tile([128, cols], BF16, tag=tag)
                        nc.sync.dma_start(
                            t[:], src[half, 128 * kb:128 * kb + 128, :])
                        lst.append(t)
                return hi, lo

            def const(name, src, shape, dt=F32):
                t = cst.tile(shape, dt, tag=name)
                nc.sync.dma_start(t[:], src)
                return t

            bq_sb = const("bq", bq8, [128, KB])
            bk_sb = const("bk", bk8, [128, KB])
            bo_sb = const("bo", bo8, [128, KB])
            bvr_sb = const("bvr", bvr, [65, D])
            bo_c = const("BOc", BOc, [64, 64], BF16)
            id_sb = const("ident", ident, [K, K])
            smk_sb = const("smk", smk, [64, 1])
            xh, xl, wqh, wql = [], [], [], []
            for kb in range(KB):
                for src, cols, pool, tag, hi_l, lo_l in (
                    (xT2, SB, xa_pool, "xa", xh, xl),
                    (wq2, D, wqo_pool, "w", wqh, wql),
                ):
                    for half, lst in ((0, hi_l), (1, lo_l)):
                        t = pool.tile([128, cols], BF16, tag=tag)
                        nc.sync.dma_start(
                            t[:], src[half, 128 * kb:128 * kb + 128, :])
                        lst.append(t)
            xs32h, xs32l = load_pair(xs32_pool, xs322, K, "xs32")
            wkh, wkl = load_pair(wk_pool, wk2, D, "wk")
            xsrh, xsrl = load_pair(xsr_pool, xsr2, 65, "xsr")

            def mm3(out, lh, ll, rh, rl, first, last):
                """out += lh.T@rh + lh.T@rl + ll.T@rh  (bf16 hi/lo 3-term)"""
                nc.tensor.matmul(out, lh, rh, start=first, stop=False)
                nc.tensor.matmul(out, lh, rl, start=False, stop=False)
                nc.tensor.matmul(out, ll, rh, start=False, stop=last)

            # ---- Q^T = wq @ x^T (+bq), bf16x3 ----
            QT = []
            for db in range(KB):
                ps = pq.tile([128, SB], F32, tag="pp")
                cs = slice(128 * db, 128 * db + 128)
                for kb in range(KB):
                    mm3(ps[:], wqh[kb][:, cs], wql[kb][:, cs], xh[kb][:],
                        xl[kb][:], kb == 0, kb == KB - 1)
                t = qtp.tile([128, SB], F32, tag="qt")
                nc.vector.tensor_scalar_add(t[:], ps[:], bq_sb[:, db:db + 1])
                QT.append(t)

            # ---- K for selected keys, bf16x3 in [j, d] layout, then PE
            # transposes build block-diagonal K^T tiles: KBD[db] is [128, 64]
            # with [0:64, 0:32] = K^T of head 2db (+bk), [64:128, 32:64] =
            # K^T of head 2db+1 (+bk), zeros elsewhere.
            Ksb = cst.tile([K, D], F32, tag="Ksb")
            for dh in range(2):
                cs = slice(512 * dh, 512 * dh + 512)
                ps = psc.tile([K, 512], F32, tag="ps")
                for kb in range(KB):
                    mm3(ps[:], xs32h[kb][:], xs32l[kb][:], wkh[kb][:, cs],
                        wkl[kb][:, cs], kb == 0, kb == KB - 1)
                nc.scalar.activation(Ksb[:, cs], ps[:], IDN)
            KBD = []
            for db in range(KB):
                pt = psc.tile([128, K], F32, tag="ps")
                nc.tensor.transpose(pt[:], Ksb[:, 128 * db:128 * db + 128],
                                    id_sb[:])
                t = kbdp.tile([128, 2 * K], F32, tag="kbd")
                nc.gpsimd.memset(t[:], 0.0)
                nc.vector.tensor_scalar_add(t[0:64, 0:K], pt[0:64, :],
                                            bk_sb[0:64, db:db + 1])
                nc.vector.tensor_scalar_add(t[64:128, K:2 * K], pt[64:128, :],
                                            bk_sb[64:128, db:db + 1])
                KBD.append(t)

            # ---- V for selected keys, 2x partition-replicated, bf16x3:
            # VR[32r+j, d] = v_sel[j, d] (+bv), r in {0,1}
            wvh, wvl = load_pair(wv_pool, wv2, D, "wv")
            VR = cst.tile([65, D], F32, tag="VR")
            for dh in range(2):
                cs = slice(512 * dh, 512 * dh + 512)
                ps = psc.tile([65, 512], F32, tag="ps")
                for kb in range(KB):
                    mm3(ps[:], xsrh[kb][:], xsrl[kb][:], wvh[kb][:, cs],
                        wvl[kb][:, cs], kb == 0, kb == KB - 1)
                nc.vector.tensor_add(VR[:, cs], ps[:], bvr_sb[:, cs])
            # row 64 of VR is V_all (the x_sum column); replicate it to
            # partitions 0 and 32 for the rank-1 matmuls
            VA = cst.tile([64, D], F32, tag="VA")
            for r in range(2):
                nc.sync.dma_start(VA[32 * r:32 * r + 1, :], VR[64:65, :])

            # ---- scores -> p-1 -> 1/denom -> weights (native fp32).
            # Tile i holds heads (2i, 2i+1): head h at partitions 32*(h%2).
            PN = []   # (p-1)/denom, fp32 [64, SB]
            IV = []   # 1/denom, fp32 (rows within a 32-group all equal)
            for i in range(KB):
                ps = psc.tile([64, SB], F32, tag="ps")
                nc.tensor.matmul(ps[:], KBD[i][:], QT[i][:], start=True, stop=True)
                p = sma.tile([64, SB], F32, tag="p")
                nc.scalar.activation(p[:], ps[:], EXP, scale=INV_SQRT_HD)
                ph = sma.tile([64, SB], BF16, tag="ph")
                nc.scalar.activation(ph[:], ps[:], EXP, scale=INV_SQRT_HD)
                pl = sma.tile([64, SB], BF16, tag="pl")
                nc.vector.tensor_sub(pl[:], p[:], ph[:])
                dn = psc.tile([64, SB], F32, tag="ps")
                nc.tensor.matmul(dn[:], bo_c[:], ph[:], start=True, stop=False)
                nc.tensor.matmul(dn[:], bo_c[:], pl[:], start=False, stop=True)
                dns = sma.tile([64, SB], F32, tag="dns")
                nc.scalar.activation(dns[:], dn[:], IDN, bias=smk_sb[:])
                iv = wv_pool.tile([64, SB], F32, tag="wv")
                nc.vector.reciprocal(iv[:], dns[:])
                pn = wv_pool.tile([64, SB], F32, tag="wv")
                nc.vector.scalar_tensor_tensor(
                    pn[:], p[:], -1.0, iv[:],
                    mybir.AluOpType.add, mybir.AluOpType.mult)
                PN.append(pn)
                IV.append(iv)

            # ---- att^T per head pair: att^T[hd, s] = V^T pn + VA/denom ----
            ATT = []  # bf16 hi/lo pairs
            for i in range(KB):  # heads (2i, 2i+1)
                ps = pa.tile([128, SB], F32, tag="pa")
                for h in (2 * i, 2 * i + 1):
                    r = 32 * (h % 2)
                    o = ps[64 * (h % 2):64 * (h % 2) + 64, :]
                    nc.tensor.matmul(
                        o, VR[r:r + 32, 64 * h:64 * h + 64],
                        PN[i][r:r + 32, :], start=True, stop=False)
                    nc.tensor.matmul(
                        o, VA[r:r + 1, 64 * h:64 * h + 64],
                        IV[i][r:r + 1, :], start=False, stop=True)
                th = xa_pool.tile([128, SB], BF16, tag="xa")
                nc.scalar.activation(th[:], ps[:], CPY)
                tl = xa_pool.tile([128, SB], BF16, tag="xa")
                nc.vector.tensor_sub(tl[:], ps[:], th[:])
                ATT.append((th, tl))

            # ---- out^T = wo @ att^T (+bo), bf16x3 ----
            woh, wol = load_pair(wqo_pool, wo2, D, "w")
            for db in range(KB):
                ps = pq.tile([128, SB], F32, tag="pp")
                cs = slice(128 * db, 128 * db + 128)
                for kb in range(KB):
                    mm3(ps[:], woh[kb][:, cs], wol[kb][:, cs], ATT[kb][0][:],
                        ATT[kb][1][:], kb == 0, kb == KB - 1)
                t = qtp.tile([128, SB], F32, tag="qt")
                nc.vector.tensor_scalar_add(t[:], ps[:], bo_sb[:, db:db + 1])
                nc.sync.dma_start(outT[128 * db:128 * db + 128, :], t[:])

    nc.compile()
    return nc


def _split(a):
    import ml_dtypes

    hi = a.astype(ml_dtypes.bfloat16)
    lo = (a - hi.astype(np.float32)).astype(ml_dtypes.bfloat16)
    return np.ascontiguousarray(np.stack([hi, lo]))


def kernel(x, wq, bq, wk, bk, wv, bv, wo, bo, ws, bs):
    global last_results
    import ml_dtypes
    from concourse.bass_utils import run_bass_kernel_spmd

    if os.environ.get("BASS_TRACE"):
        _install_ntff_hook()

    x = np.asarray(x, dtype=np.float32)
    wqT, wkT, wvT, woT = (np.ascontiguousarray(np.asarray(w, np.float32).T)
                          for w in (wq, wk, wv, wo))
    bq, bk, bv, bo = (np.asarray(b, np.float32) for b in (bq, bk, bv, bo))
    ws = np.asarray(ws, np.float32)
    bs = np.float32(bs)

    # host prep: importance -> top-K selection -> gather (GEMV + argpartition)
    imp = x.astype(np.float64) @ ws.astype(np.float64) + float(bs)   # [B,S]
    top = np.sort(np.argpartition(imp, S - K, axis=1)[:, -K:], axis=1)
    xsel = np.take_along_axis(x, top[:, :, None], axis=1)            # [B,K,D]
    xsum = x.astype(np.float64).sum(axis=1).astype(np.float32)       # [B,D]

    if "nc" not in _cache:
        _cache["nc"] = _build()
    nc = _cache["nc"]

    blockones = np.zeros((64, 64), np.float32)
    for g in range(2):
        blockones[32 * g:32 * g + 32, 32 * g:32 * g + 32] = 1.0

    def btile(b_):
        return np.ascontiguousarray(b_.reshape(KB, 128).T)

    wq2, wv2, wo2, wk2 = _split(wqT), _split(wvT), _split(woT), _split(wkT)
    in_maps = []
    for c in range(NC):
        b = c // CPB
        r0 = (c % CPB) * SB
        xsT = np.ascontiguousarray(xsel[b].T)                        # [D, K]
        in_maps.append({
            "xT2": _split(np.ascontiguousarray(x[b, r0:r0 + SB, :].T)),
            "wq2": wq2, "wv2": wv2, "wo2": wo2, "wk2": wk2,
            "xs322": _split(xsT),
            "xsr2": _split(np.concatenate(
                [np.tile(xsT, (1, 2)), xsum[b][:, None]], axis=1)),
            "bq8": btile(bq), "bk8": btile(bk), "bo8": btile(bo),
            "bvr": np.ascontiguousarray(np.concatenate(
                [np.tile(bv[None, :], (64, 1)), float(S) * bv[None, :]])),
            "BOc": blockones.astype(ml_dtypes.bfloat16),
            "ident": np.eye(K, dtype=np.float32),
            "smk": np.full((64, 1), float(S - K), np.float32),
        })

    res = run_bass_kernel_spmd(nc, in_maps, list(range(NC)))
    last_results = res

    out = np.empty((B, S, D), np.float32)
    for c in range(NC):
        b = c // CPB
        r0 = (c % CPB) * SB
        out[b, r0:r0 + SB, :] = res.results[c]["outT"].T
    return out
